# revision 1
# baseline (speedup 1.0000x reference)
"""LoRA Linear kernel for Trainium2, 8 NeuronCores, data-parallel over tokens.

out = x @ W^T + bias + 2.0 * (x @ A^T) @ B^T
  x: [4, 2048, 4096] f32, W: [4096, 4096], bias: [4096], A: [16, 4096], B: [4096, 16]

Strategy:
  - Flatten tokens (8192) and shard 1024 tokens per core (pure data parallel,
    no collectives; gather on host).
  - Host pre-transposes so the contraction dim d lands on SBUF partitions:
      xt = x_shard^T [4096, 1024], wt = W^T [4096, 4096], at = A^T [4096, 16].
  - Each core computes out^T [4096, 1024]: for each (o-tile 128, m-chunk 512)
    PSUM tile, accumulate 32 float32r matmuls over d (W stationary), then one
    extra K=128 matmul adds the LoRA update AND the bias:
      ub rows 0..15 = (2*B)^T, row 16 = bias, rows 17..127 = 0
      xab rows 0..15 = xa^T = A x^T, row 16 = ones, rows 17..127 = 0
  - float32r matmuls run at 1 cycle/row for N>=256 (4x faster than float32).
  - Host transposes/concats the 8 out^T shards back to [4, 2048, 4096].
"""

import sys
from contextlib import ExitStack

import numpy as np

sys.path.insert(0, "/opt/trn_rl_repo")

import concourse.bacc as bacc  # noqa: E402
import concourse.bass as bass  # noqa: E402
import concourse.mybir as mybir  # noqa: E402
import concourse.tile as tile  # noqa: E402
from concourse.bass import ts  # noqa: E402
from concourse.bass_utils import run_bass_kernel_spmd  # noqa: E402

P = 128
B_DIM, S_DIM = 4, 2048
D = 4096          # in_features (contraction)
O = 4096          # out_features
R = 16            # lora rank
SCALING = 2.0     # alpha / rank = 32/16
NCORES = 8
M = (B_DIM * S_DIM) // NCORES   # tokens per core = 1024
KD = D // P       # 32 contraction tiles
MC = 512          # moving free dim per matmul
NMC = M // MC     # 2 m-chunks
NO = O // P       # 32 output-feature tiles

FR = mybir.dt.float32r
F32 = mybir.dt.float32


def build_program() -> bass.Bass:
    # Bacc (not plain Bass): its compile() pipeline splits multi-wait
    # matmuls via event semaphores — walrus allows at most one sync wait
    # on a self-loading f32r matmul.
    nc = bacc.Bacc()
    xt = nc.dram_tensor("xt", [D, M], FR, kind="ExternalInput")
    wt = nc.dram_tensor("wt", [D, O], FR, kind="ExternalInput")
    at = nc.dram_tensor("at", [D, R], FR, kind="ExternalInput")
    # ub: rows 0..15 = (2*lora_b)^T, row 16 = bias, rows 17..127 = 0
    ub = nc.dram_tensor("ub", [P, O], FR, kind="ExternalInput")
    # fill for xab rows 16..127: row 16 = ones, rest zeros
    fill = nc.dram_tensor("fill", [P - R, NMC, MC], FR, kind="ExternalInput")
    outT = nc.dram_tensor("outT", [O, M], F32, kind="ExternalOutput")

    xt_r = xt.rearrange("(ko p) m -> p ko m", p=P)   # [128, 32, 1024]
    at_r = at.rearrange("(ko p) r -> p ko r", p=P)   # [128, 32, 16]
    wt_r = wt.rearrange("(ko p) o -> p ko o", p=P)   # [128, 32, 4096]

    with ExitStack() as ctx:
        tc = ctx.enter_context(tile.TileContext(nc))
        xt_pool = ctx.enter_context(tc.tile_pool(name="xtp", bufs=1))
        cpool = ctx.enter_context(tc.tile_pool(name="cpool", bufs=1))
        wt_pool = ctx.enter_context(tc.tile_pool(name="wtp", bufs=2))
        out_pool = ctx.enter_context(tc.tile_pool(name="outp", bufs=4))
        ps_pool = ctx.enter_context(tc.tile_pool(name="psp", bufs=4, space="PSUM"))
        psxa_pool = ctx.enter_context(tc.tile_pool(name="psxa", bufs=2, space="PSUM"))

        xt_sb = xt_pool.tile([P, KD, M], FR)
        at_sb = cpool.tile([P, KD, R], FR)
        ub_sb = cpool.tile([P, O], FR)            # rows 0..16 real, rest zero
        xab_sb = cpool.tile([P, NMC, MC], FR)     # rows 0..16 real, rest zero

        # Constant loads on the ACT HWDGE ring; weight stream on the SP ring.
        # (memset can't produce f32r, so all constant fills come via DMA.)
        nc.scalar.dma_start(at_sb[:], at_r)
        nc.scalar.dma_start(ub_sb[:], ub[:])
        nc.scalar.dma_start(xab_sb[R:P, :, :], fill[:])
        # Split the x^T load along the contraction dim so the first matmuls
        # (which consume ko-chunk 0) start ~18us earlier than one bulk DMA.
        XSPLIT = 4
        kchunk = KD // XSPLIT
        for mi in range(NMC):
            for h in range(XSPLIT):
                nc.scalar.dma_start(
                    xt_sb[:, ts(h, kchunk), ts(mi, MC)],
                    xt_r[:, ts(h, kchunk), ts(mi, MC)],
                )

        # xa^T[r, m] = sum_d A^T[d, r]^T x^T[d, m] for each m-chunk
        for mi in range(NMC):
            ps_xa = psxa_pool.tile([R, MC], F32)
            for k in range(KD):
                nc.tensor.matmul(
                    ps_xa[:],
                    lhsT=at_sb[:, k, :],
                    rhs=xt_sb[:, k, ts(mi, MC)],
                    start=(k == 0),
                    stop=(k == KD - 1),
                )
            # out dtype f32r => DVE rounds to the 20-bit fp32r format, as the
            # BIR verifier requires for matmul operands.
            nc.vector.tensor_copy(out=xab_sb[0:R, mi, :], in_=ps_xa[:])

        # Main: out^T tile [o=128, m=512] = W-block^T.T @ x^T + ub^T.T @ xab
        for oi in range(NO):
            wt_sb = wt_pool.tile([P, KD, P], FR)
            nc.sync.dma_start(wt_sb[:], wt_r[:, :, ts(oi, P)])
            for mi in range(NMC):
                ps = ps_pool.tile([P, MC], F32)
                for k in range(KD):
                    nc.tensor.matmul(
                        ps[:],
                        lhsT=wt_sb[:, k, :],
                        rhs=xt_sb[:, k, ts(mi, MC)],
                        start=(k == 0),
                        stop=False,
                    )
                # rank-16 LoRA update + bias (via the ones row), zero-padded to K=128
                nc.tensor.matmul(
                    ps[:],
                    lhsT=ub_sb[:, ts(oi, P)],
                    rhs=xab_sb[:, mi, :],
                    start=False,
                    stop=True,
                )
                ot = out_pool.tile([P, MC], F32)
                nc.vector.tensor_copy(out=ot[:], in_=ps[:])
                nc.gpsimd.dma_start(outT[ts(oi, P), ts(mi, MC)], ot[:])
    nc.compile()
    return nc


def round_f32r(a: np.ndarray) -> np.ndarray:
    """Round-to-nearest-even into the fp32r format (1s + 8e + 11m, low 12
    bits zero). Matmul operands must be pre-rounded for well-defined HW
    behavior; the on-chip producers round, so round host inputs too."""
    b = np.ascontiguousarray(a, dtype=np.float32).view(np.uint32)
    lsb = (b >> np.uint32(12)) & np.uint32(1)
    r = (b + np.uint32(0x07FF) + lsb) & np.uint32(0xFFFFF000)
    return r.view(np.float32)


def prepare_in_maps(inputs, weight, bias, lora_a, lora_b):
    x = round_f32r(
        np.ascontiguousarray(np.asarray(inputs, dtype=np.float32)).reshape(
            B_DIM * S_DIM, D
        )
    )
    wt = round_f32r(np.ascontiguousarray(np.asarray(weight, dtype=np.float32).T))
    at = round_f32r(np.ascontiguousarray(np.asarray(lora_a, dtype=np.float32).T))
    ub = round_f32r(
        np.concatenate(
            [
                SCALING * np.asarray(lora_b, dtype=np.float32).T,
                np.asarray(bias, dtype=np.float32)[None, :],
                np.zeros((P - R - 1, O), dtype=np.float32),
            ],
            axis=0,
        ).astype(np.float32)
    )
    fill = np.zeros((P - R, NMC, MC), dtype=np.float32)
    fill[0] = 1.0
    in_maps = []
    for c in range(NCORES):
        xt_c = np.ascontiguousarray(x[c * M : (c + 1) * M].T)
        in_maps.append({"xt": xt_c, "wt": wt, "at": at, "ub": ub, "fill": fill})
    return in_maps


def run(inputs, weight, bias, lora_a, lora_b, trace=False):
    nc = build_program()
    in_maps = prepare_in_maps(inputs, weight, bias, lora_a, lora_b)
    res = run_bass_kernel_spmd(nc, in_maps, list(range(NCORES)), trace=trace)
    shards = [np.asarray(res.results[c]["outT"]).T for c in range(NCORES)]
    out = np.concatenate(shards, axis=0).reshape(B_DIM, S_DIM, O)
    return np.ascontiguousarray(out, dtype=np.float32), res


def kernel(inputs, weight, bias, lora_a, lora_b):
    out, _ = run(inputs, weight, bias, lora_a, lora_b, trace=False)
    return out



# revision 12
# speedup vs baseline: 1.3533x; 1.3533x over previous
"""LoRA Linear kernel for Trainium2, 8 NeuronCores, data-parallel over tokens.

out = x @ W^T + bias + 2.0 * (x @ A^T) @ B^T
  x: [4, 2048, 4096] f32, W: [4096, 4096], bias: [4096], A: [16, 4096], B: [4096, 16]

Strategy:
  - Fold the LoRA update into the weight on the host: W' = W + 2.0 * (B @ A)
    (mathematically identical), so the device runs a single dense GEMM + bias.
  - Flatten tokens (8192) and shard 1024 tokens per core (pure data parallel,
    no collectives; gather on host).
  - bf16 operands: same 1 cycle/row PE speed as f32r but half the HBM traffic
    (rel err ~2e-3 vs the 2e-2 gate). PSUM accumulation stays f32.
  - x^T blocks are the stationary operand, W'^T slices the moving one, so the
    output lands untransposed as [tokens, features].
  - The contraction dim d maps to partitions p-major (d = p*32 + ko), which
    makes every DMA's per-partition HBM runs 4-8KB contiguous (the ko-major
    mapping gives 1-2KB runs and ~half DMA throughput). wt is additionally
    host-blocked per o-chunk so its slices are contiguous too.
  - Pass structure over 8 o-chunks of 512, with 8 PSUM banks:
      pass 0: k-outer (for k: for mt) so the PE consumes x^T chunks in DMA
        arrival order - compute starts ~10us in instead of stalling ~40us
        for the full x^T load.
      pass 1: k-outer with a skewed (anti-diagonal) entry, so each PSUM bank
        is first touched only after pass 0's staggered DVE drain frees it.
      passes 2-7: mt-outer/k-inner, so banks close 6.9us apart and the DVE
        bias-add drain fully overlaps compute (no pass-boundary PE stall,
        which would also re-throttle the HAM clock gate).
  - Bias is added by the (otherwise idle) DVE during PSUM->SBUF copy-out.
  - A few zero matmuls up front warm the PE's HAM clock gate (1.2 -> 2.4 GHz)
    while the first DMA chunks are still in flight.
"""

import sys
from contextlib import ExitStack

import numpy as np
import ml_dtypes

sys.path.insert(0, "/opt/trn_rl_repo")

import concourse.bacc as bacc  # noqa: E402
import concourse.bass as bass  # noqa: E402
import concourse.mybir as mybir  # noqa: E402
import concourse.tile as tile  # noqa: E402
from concourse.bass import ts  # noqa: E402
from concourse.bass_utils import run_bass_kernel_spmd  # noqa: E402

P = 128
B_DIM, S_DIM = 4, 2048
D = 4096          # in_features (contraction)
O = 4096          # out_features
SCALING = 2.0     # alpha / rank = 32/16
NCORES = 8
M = (B_DIM * S_DIM) // NCORES   # tokens per core = 1024
KD = D // P       # 32 contraction tiles
OC = 512          # out-feature chunk (moving free dim per matmul)
NOC = O // OC     # 8 o-chunks
NMT = M // P      # 8 token tiles
NWARM = 8         # HAM warm-up matmuls

BF16 = mybir.dt.bfloat16
F32 = mybir.dt.float32


def pass_order(pass_idx: int):
    """(k, mt) emission order for one o-chunk pass."""
    if pass_idx == 0:
        return [(k, mt) for k in range(KD) for mt in range(NMT)]
    if pass_idx == 1:
        # anti-diagonal: bank mt first touched at diagonal mt, matching the
        # rate at which pass 0's DVE drain frees banks
        order = []
        for s in range(KD + NMT - 1):
            for mt in range(NMT):
                k = s - mt
                if 0 <= k < KD:
                    order.append((k, mt))
        return order
    return [(k, mt) for mt in range(NMT) for k in range(KD)]


def build_program() -> bass.Bass:
    # Bacc (not plain Bass): its compile() pipeline splits multi-wait
    # matmuls via event semaphores.
    nc = bacc.Bacc()
    xt = nc.dram_tensor("xt", [D, M], BF16, kind="ExternalInput")
    # host-blocked: [noc*128, 32, 512]; rows (oc*128+p) hold d=p*32+ko
    wt = nc.dram_tensor("wt", [NOC * P, KD, OC], BF16, kind="ExternalInput")
    biasb = nc.dram_tensor("biasb", [P, O], F32, kind="ExternalInput")
    out = nc.dram_tensor("out", [M, O], F32, kind="ExternalOutput")

    xt_r = xt.rearrange("(p ko) m -> p ko m", ko=KD)   # [128, 32, 1024]

    with ExitStack() as ctx:
        tc = ctx.enter_context(tile.TileContext(nc))
        xt_pool = ctx.enter_context(tc.tile_pool(name="xtp", bufs=1))
        cpool = ctx.enter_context(tc.tile_pool(name="cpool", bufs=1))
        wt_pool = ctx.enter_context(tc.tile_pool(name="wtp", bufs=2))
        out_pool = ctx.enter_context(tc.tile_pool(name="outp", bufs=8))
        ps_pool = ctx.enter_context(tc.tile_pool(name="psp", bufs=1, space="PSUM"))

        xt_sb = xt_pool.tile([P, KD, M], BF16)       # 64 KB/partition
        bias_sb = cpool.tile([P, O], F32)            # 16 KB/partition
        wmA = cpool.tile([P, P], BF16)
        wmB = cpool.tile([P, OC], BF16)
        ps = [ps_pool.tile([P, OC], F32, name=f"ps_{i}") for i in range(NMT)]

        # HAM warm-up: zero matmuls keep the PE busy (and un-throttled)
        # while the first real DMA chunks land. Memsets go on the vector
        # queue so the gpsimd ring can start posting x^T chunks immediately.
        nc.vector.memset(wmA[:], 0.0)
        nc.vector.memset(wmB[:], 0.0)
        for w in range(NWARM):
            nc.tensor.matmul(
                ps[w][:], lhsT=wmA[:], rhs=wmB[:], start=True, stop=True
            )

        # x^T k-chunks alternate between the scalar and gpsimd rings; the
        # k-outer pass-0 loop consumes them in arrival order. The first two
        # chunks are single k-slices so the PE can start under the 8-core
        # HBM contention burst at kernel start.
        xsplits = [1, 1] + [2] * 15
        k0 = 0
        for h, xk in enumerate(xsplits):
            eng = nc.scalar if h % 2 == 0 else nc.gpsimd
            eng.dma_start(
                xt_sb[:, k0 : k0 + xk, :], xt_r[:, k0 : k0 + xk, :]
            )
            k0 += xk
        # bias rides the gpsimd ring behind the x^T chunks (it is not
        # needed until the first copy-out ~60us in)
        nc.gpsimd.dma_start(bias_sb[:], biasb[:])

        for oc in range(NOC):
            wt_sb = wt_pool.tile([P, KD, OC], BF16)  # 32 KB/partition
            # pass 0 consumes wt in k-order as it streams, so split finely
            # (leading chunks smallest); later passes are prefetched a full
            # pass ahead - coarser chunks mean fewer PE wait-events (each
            # satisfied wait still breaks back-to-back matmul pipelining)
            if oc == 0:
                wsplits = [2, 2, 4, 4, 4, 4, 4, 4, 4]
            elif oc == 1:
                wsplits = [4] * 8
            else:
                wsplits = [16, 16]
            k0 = 0
            for wk in wsplits:
                nc.sync.dma_start(
                    wt_sb[:, k0 : k0 + wk, :], wt[ts(oc, P), k0 : k0 + wk, :]
                )
                k0 += wk
            for k, mt in pass_order(oc):
                nc.tensor.matmul(
                    ps[mt][:],
                    lhsT=xt_sb[:, k, ts(mt, P)],
                    rhs=wt_sb[:, k, :],
                    start=(k == 0),
                    stop=(k == KD - 1),
                )
            for mt in range(NMT):
                ot = out_pool.tile([P, OC], F32)
                nc.vector.tensor_tensor(
                    ot[:], ps[mt][:], bias_sb[:, ts(oc, OC)], mybir.AluOpType.add
                )
                # spread the final pass's drain across all three rings
                if oc == NOC - 1:
                    eng = (nc.gpsimd, nc.scalar, nc.sync)[mt % 3]
                else:
                    eng = nc.gpsimd
                eng.dma_start(out[ts(mt, P), ts(oc, OC)], ot[:])
    nc.compile()
    return nc


def prepare_in_maps(inputs, weight, bias, lora_a, lora_b):
    x = np.ascontiguousarray(np.asarray(inputs, dtype=np.float32)).reshape(
        B_DIM * S_DIM, D
    )
    w_folded = np.asarray(weight, dtype=np.float32) + SCALING * (
        np.asarray(lora_b, dtype=np.float32) @ np.asarray(lora_a, dtype=np.float32)
    )
    # [D, O] -> [NOC, P, KD, OC] with d = p*KD + ko, then flatten the first two
    wt = np.ascontiguousarray(
        w_folded.T.reshape(P, KD, NOC, OC).transpose(2, 0, 1, 3).reshape(
            NOC * P, KD, OC
        )
    ).astype(ml_dtypes.bfloat16)
    biasb = np.ascontiguousarray(
        np.tile(np.asarray(bias, dtype=np.float32)[None, :], (P, 1))
    )
    in_maps = []
    for c in range(NCORES):
        xt_c = np.ascontiguousarray(x[c * M : (c + 1) * M].T).astype(
            ml_dtypes.bfloat16
        )
        in_maps.append({"xt": xt_c, "wt": wt, "biasb": biasb})
    return in_maps


def run(inputs, weight, bias, lora_a, lora_b, trace=False):
    nc = build_program()
    in_maps = prepare_in_maps(inputs, weight, bias, lora_a, lora_b)
    res = run_bass_kernel_spmd(nc, in_maps, list(range(NCORES)), trace=trace)
    shards = [np.asarray(res.results[c]["out"]) for c in range(NCORES)]
    out = np.concatenate(shards, axis=0).reshape(B_DIM, S_DIM, O)
    return np.ascontiguousarray(out, dtype=np.float32), res


def kernel(inputs, weight, bias, lora_a, lora_b):
    out, _ = run(inputs, weight, bias, lora_a, lora_b, trace=False)
    return out


# revision 14
# speedup vs baseline: 1.3583x; 1.0037x over previous
"""LoRA Linear kernel for Trainium2, 8 NeuronCores, data-parallel over tokens.

out = x @ W^T + bias + 2.0 * (x @ A^T) @ B^T
  x: [4, 2048, 4096] f32, W: [4096, 4096], bias: [4096], A: [16, 4096], B: [4096, 16]

Strategy:
  - Fold the LoRA update into the weight on the host: W' = W + 2.0 * (B @ A)
    (mathematically identical), so the device runs a single dense GEMM + bias.
  - Flatten tokens (8192) and shard 1024 tokens per core (pure data parallel,
    no collectives; gather on host).
  - bf16 operands: same 1 cycle/row PE speed as f32r but half the HBM traffic
    (rel err ~2e-3 vs the 2e-2 gate). PSUM accumulation stays f32.
  - x^T blocks are the stationary operand, W'^T slices the moving one, so the
    output lands untransposed as [tokens, features].
  - The contraction dim d maps to partitions p-major (d = p*32 + ko), which
    makes every DMA's per-partition HBM runs 4-8KB contiguous (the ko-major
    mapping gives 1-2KB runs and ~half DMA throughput). wt is additionally
    host-blocked per o-chunk so its slices are contiguous too.
  - Pass structure over 8 o-chunks of 512, with 8 PSUM banks:
      pass 0: k-outer (for k: for mt) so the PE consumes x^T chunks in DMA
        arrival order - compute starts ~10us in instead of stalling ~40us
        for the full x^T load.
      pass 1: k-outer with a skewed (anti-diagonal) entry, so each PSUM bank
        is first touched only after pass 0's staggered DVE drain frees it.
      passes 2-7: mt-outer/k-inner, so banks close 6.9us apart and the DVE
        bias-add drain fully overlaps compute (no pass-boundary PE stall,
        which would also re-throttle the HAM clock gate).
  - Bias is added by the (otherwise idle) DVE during PSUM->SBUF copy-out.
  - A few zero matmuls up front warm the PE's HAM clock gate (1.2 -> 2.4 GHz)
    while the first DMA chunks are still in flight.
"""

import sys
from contextlib import ExitStack

import numpy as np
import ml_dtypes

sys.path.insert(0, "/opt/trn_rl_repo")

import concourse.bacc as bacc  # noqa: E402
import concourse.bass as bass  # noqa: E402
import concourse.mybir as mybir  # noqa: E402
import concourse.tile as tile  # noqa: E402
from concourse.bass import ts  # noqa: E402
from concourse.bass_utils import run_bass_kernel_spmd  # noqa: E402

P = 128
B_DIM, S_DIM = 4, 2048
D = 4096          # in_features (contraction)
O = 4096          # out_features
SCALING = 2.0     # alpha / rank = 32/16
NCORES = 8
M = (B_DIM * S_DIM) // NCORES   # tokens per core = 1024
KD = D // P       # 32 contraction tiles
OC = 512          # out-feature chunk (moving free dim per matmul)
NOC = O // OC     # 8 o-chunks
NMT = M // P      # 8 token tiles
NWARM = 12        # HAM warm-up matmuls (bridge the PE to first-data arrival)

BF16 = mybir.dt.bfloat16
F32 = mybir.dt.float32


def pass_order(pass_idx: int):
    """(k, mt) emission order for one o-chunk pass."""
    if pass_idx == 0:
        return [(k, mt) for k in range(KD) for mt in range(NMT)]
    if pass_idx == 1:
        # anti-diagonal: bank mt first touched at diagonal mt, matching the
        # rate at which pass 0's DVE drain frees banks
        order = []
        for s in range(KD + NMT - 1):
            for mt in range(NMT):
                k = s - mt
                if 0 <= k < KD:
                    order.append((k, mt))
        return order
    return [(k, mt) for mt in range(NMT) for k in range(KD)]


def build_program() -> bass.Bass:
    # Bacc (not plain Bass): its compile() pipeline splits multi-wait
    # matmuls via event semaphores.
    nc = bacc.Bacc()
    xt = nc.dram_tensor("xt", [D, M], BF16, kind="ExternalInput")
    # host-blocked: [noc*128, 32, 512]; rows (oc*128+p) hold d=p*32+ko
    wt = nc.dram_tensor("wt", [NOC * P, KD, OC], BF16, kind="ExternalInput")
    biasb = nc.dram_tensor("biasb", [P, O], F32, kind="ExternalInput")
    out = nc.dram_tensor("out", [M, O], F32, kind="ExternalOutput")

    xt_r = xt.rearrange("(p ko) m -> p ko m", ko=KD)   # [128, 32, 1024]

    with ExitStack() as ctx:
        tc = ctx.enter_context(tile.TileContext(nc))
        xt_pool = ctx.enter_context(tc.tile_pool(name="xtp", bufs=1))
        cpool = ctx.enter_context(tc.tile_pool(name="cpool", bufs=1))
        wt_pool = ctx.enter_context(tc.tile_pool(name="wtp", bufs=2))
        out_pool = ctx.enter_context(tc.tile_pool(name="outp", bufs=8))
        ps_pool = ctx.enter_context(tc.tile_pool(name="psp", bufs=1, space="PSUM"))

        xt_sb = xt_pool.tile([P, KD, M], BF16)       # 64 KB/partition
        bias_sb = cpool.tile([P, O], F32)            # 16 KB/partition
        wmA = cpool.tile([P, P], BF16)
        wmB = cpool.tile([P, OC], BF16)
        ps = [ps_pool.tile([P, OC], F32, name=f"ps_{i}") for i in range(NMT)]

        # HAM warm-up: zero matmuls keep the PE busy (and un-throttled)
        # while the first real DMA chunks land. Memsets go on the vector
        # queue so the gpsimd ring can start posting x^T chunks immediately.
        nc.vector.memset(wmA[:], 0.0)
        nc.vector.memset(wmB[:], 0.0)
        for w in range(NWARM):
            nc.tensor.matmul(
                ps[w % NMT][:], lhsT=wmA[:], rhs=wmB[:], start=True, stop=True
            )

        # x^T k-chunks alternate between the scalar and gpsimd rings; the
        # k-outer pass-0 loop consumes them in arrival order. The first two
        # chunks are single k-slices so the PE can start under the 8-core
        # HBM contention burst at kernel start.
        xsplits = [1, 1] + [2] * 15
        k0 = 0
        for h, xk in enumerate(xsplits):
            eng = nc.scalar if h % 2 == 0 else nc.gpsimd
            eng.dma_start(
                xt_sb[:, k0 : k0 + xk, :], xt_r[:, k0 : k0 + xk, :]
            )
            k0 += xk
        # bias rides the gpsimd ring behind the x^T chunks (it is not
        # needed until the first copy-out ~60us in)
        nc.gpsimd.dma_start(bias_sb[:], biasb[:])

        for oc in range(NOC):
            wt_sb = wt_pool.tile([P, KD, OC], BF16)  # 32 KB/partition
            # pass 0 consumes wt in k-order as it streams, so split finely
            # (leading chunks smallest); later passes are prefetched a full
            # pass ahead - coarser chunks mean fewer PE wait-events (each
            # satisfied wait still breaks back-to-back matmul pipelining)
            if oc == 0:
                wsplits = [2, 2, 4, 4, 4, 4, 4, 4, 4]
            elif oc == 1:
                wsplits = [4] * 8
            else:
                wsplits = [16, 16]
            k0 = 0
            for wk in wsplits:
                nc.sync.dma_start(
                    wt_sb[:, k0 : k0 + wk, :], wt[ts(oc, P), k0 : k0 + wk, :]
                )
                k0 += wk
            for k, mt in pass_order(oc):
                nc.tensor.matmul(
                    ps[mt][:],
                    lhsT=xt_sb[:, k, ts(mt, P)],
                    rhs=wt_sb[:, k, :],
                    start=(k == 0),
                    stop=(k == KD - 1),
                )
            for mt in range(NMT):
                ot = out_pool.tile([P, OC], F32)
                nc.vector.tensor_tensor(
                    ot[:], ps[mt][:], bias_sb[:, ts(oc, OC)], mybir.AluOpType.add
                )
                # spread the final pass's drain across all three rings
                if oc == NOC - 1:
                    eng = (nc.gpsimd, nc.scalar, nc.sync)[mt % 3]
                else:
                    eng = nc.gpsimd
                eng.dma_start(out[ts(mt, P), ts(oc, OC)], ot[:])
    nc.compile()
    return nc


def prepare_in_maps(inputs, weight, bias, lora_a, lora_b):
    x = np.ascontiguousarray(np.asarray(inputs, dtype=np.float32)).reshape(
        B_DIM * S_DIM, D
    )
    w_folded = np.asarray(weight, dtype=np.float32) + SCALING * (
        np.asarray(lora_b, dtype=np.float32) @ np.asarray(lora_a, dtype=np.float32)
    )
    # [D, O] -> [NOC, P, KD, OC] with d = p*KD + ko, then flatten the first two
    wt = np.ascontiguousarray(
        w_folded.T.reshape(P, KD, NOC, OC).transpose(2, 0, 1, 3).reshape(
            NOC * P, KD, OC
        )
    ).astype(ml_dtypes.bfloat16)
    biasb = np.ascontiguousarray(
        np.tile(np.asarray(bias, dtype=np.float32)[None, :], (P, 1))
    )
    in_maps = []
    for c in range(NCORES):
        xt_c = np.ascontiguousarray(x[c * M : (c + 1) * M].T).astype(
            ml_dtypes.bfloat16
        )
        in_maps.append({"xt": xt_c, "wt": wt, "biasb": biasb})
    return in_maps


def run(inputs, weight, bias, lora_a, lora_b, trace=False):
    nc = build_program()
    in_maps = prepare_in_maps(inputs, weight, bias, lora_a, lora_b)
    res = run_bass_kernel_spmd(nc, in_maps, list(range(NCORES)), trace=trace)
    shards = [np.asarray(res.results[c]["out"]) for c in range(NCORES)]
    out = np.concatenate(shards, axis=0).reshape(B_DIM, S_DIM, O)
    return np.ascontiguousarray(out, dtype=np.float32), res


def kernel(inputs, weight, bias, lora_a, lora_b):
    out, _ = run(inputs, weight, bias, lora_a, lora_b, trace=False)
    return out
